# revision 6
# baseline (speedup 1.0000x reference)
"""MC-CNN accurate-architecture stereo matching cost kernel for Trainium2.

Strategy: data-parallel over image rows across 8 NeuronCores. Each core
computes a 7-row band of the (54, 118, 9) output:
  - conv features for its 17-row input window (7 output rows + 10 halo),
    both images, 5 layers, via tap-accumulated matmuls,
  - the 224->384->384->384->1 MLP for all 9 disparities over its band,
    with disparity shifts realized as SBUF column offsets (no gather),
  - sigmoid on the scalar engine; negation + NaN masking on host.

All matmuls run in bf16 (fp32 PSUM accumulation; ~8e-4 rel error vs the
fp32 reference, simulated host-side). bf16 weights with M=128 enable the
FWL fast weight-load path. Biases are fused into the PSUM-evacuation op
(ACT relu / DVE add+max), alternating engines to balance load.
"""

import ml_dtypes
import numpy as np

BF16 = ml_dtypes.bfloat16

N_CORES = 8
D = 9                 # disparities -8..0
HF, WF = 54, 118
RPC = 7               # output rows per core
HALO = 17             # input rows per core window
ROW_STARTS = [min(RPC * i, HF - RPC) for i in range(N_CORES)]

# conv pipeline geometry (per core): computed rows padded to multiples of 4
L_ROWS_CMP = [16, 16, 12, 12, 8]
L_ROWS_ALL = [18, 16, 14, 12, 8]
L_W = [126, 124, 122, 120, 118]

_prog_cache = {}


def _build_program():
    import concourse.bacc as bacc
    import concourse.mybir as mybir
    import concourse.tile as tile

    f32 = mybir.dt.float32
    bf = mybir.dt.bfloat16
    AF = mybir.ActivationFunctionType
    ALU = mybir.AluOpType

    nc = bacc.Bacc("TRN2", target_bir_lowering=False, debug=False)

    img9_d = nc.declare_dram_parameter("img9", [9, 2, 16, 128], bf, False)
    cw1_d = nc.declare_dram_parameter("cw1", [9, 128], bf, False)
    cw_d = [nc.declare_dram_parameter(f"cw{l}", [112, 9, 128], bf, False)
            for l in (2, 3, 4, 5)]
    fw1_d = nc.declare_dram_parameter("fw1", [112, 2, 384], bf, False)
    fw2_d = nc.declare_dram_parameter("fw2", [128, 3, 384], bf, False)
    fw3_d = nc.declare_dram_parameter("fw3", [128, 3, 384], bf, False)
    fw4_d = nc.declare_dram_parameter("fw4", [128, 3], bf, False)
    bia_d = nc.declare_dram_parameter("biases", [128, 14], f32, False)
    zer_d = nc.declare_dram_parameter("zeros", [112, 256], bf, False)
    fb4_d = nc.declare_dram_parameter("fb4", [1, 1], f32, False)
    out_d = nc.declare_dram_parameter("out", [9, 826], f32, True)

    evac_ctr = [0]

    with tile.TileContext(nc) as tc:
        with tc.tile_pool(name="w", bufs=1) as wp, \
             tc.tile_pool(name="feat", bufs=1) as fp, \
             tc.tile_pool(name="act", bufs=3) as apool, \
             tc.tile_pool(name="stg", bufs=3) as stpool, \
             tc.tile_pool(name="ps", bufs=6, space="PSUM") as psp, \
             tc.tile_pool(name="p4", bufs=2, space="PSUM") as p4p:

            # ---- input / weight DMAs (conv-phase tensors first) ----
            img9s = wp.tile([9, 2, 16, 128], bf)
            nc.sync.dma_start(out=img9s, in_=img9_d[:, :, :, :])
            cw1s = wp.tile([9, 128], bf)
            nc.sync.dma_start(out=cw1s, in_=cw1_d[:, :])
            cws = []
            for i in range(4):
                t = wp.tile([112, 9, 128], bf, name=f"cw{i + 2}s")
                # per-tap DMAs so tap 0 lands early and layer l+2 can start
                # as soon as its first weights arrive
                for tap in range(9):
                    nc.sync.dma_start(out=t[:, tap, :],
                                      in_=cw_d[i][:, tap, :])
                cws.append(t)
            bias = wp.tile([128, 14], f32)
            nc.sync.dma_start(out=bias, in_=bia_d[:, :])
            fw1s = wp.tile([112, 2, 384], bf)
            nc.sync.dma_start(out=fw1s, in_=fw1_d[:, :, :])
            fw2s = wp.tile([128, 3, 384], bf)
            nc.sync.dma_start(out=fw2s, in_=fw2_d[:, :, :])
            fw3s = wp.tile([128, 3, 384], bf)
            nc.sync.dma_start(out=fw3s, in_=fw3_d[:, :, :])
            fw4s = wp.tile([128, 3], bf)
            nc.sync.dma_start(out=fw4s, in_=fw4_d[:, :])
            fb4s = wp.tile([1, 1], f32)
            nc.sync.dma_start(out=fb4s, in_=fb4_d[:, :])

            # ---- feature tiles ----
            feats = []  # feats[img][layer]
            for i in range(2):
                per = []
                for l in range(5):
                    if l < 4:
                        t = fp.tile([112, L_ROWS_ALL[l], 128], bf,
                                    name=f"f{l + 1}_{i}")
                    else:
                        t = fp.tile([112, 8, 136], bf, name=f"f5_{i}")
                    per.append(t)
                feats.append(per)
                # zero regions that are read before/never being written
                z2 = zer_d[:, :].rearrange("p (a b) -> p a b", a=2)
                z8 = zer_d[:, 0:64].rearrange("p (a b) -> p a b", a=8)
                nc.sync.dma_start(out=per[0][:, 16:18, :], in_=z2)
                nc.sync.dma_start(out=per[2][:, 12:14, :], in_=z2)
                nc.sync.dma_start(out=per[4][:, :, 0:8], in_=z8)

            def evac_relu(ps, out_ap, bias_ap):
                # relu(x + b), PSUM -> SBUF, alternating ACT / DVE
                if evac_ctr[0] % 2 == 0:
                    nc.scalar.activation(out=out_ap, in_=ps, func=AF.Relu,
                                         bias=bias_ap)
                else:
                    nc.vector.tensor_scalar(out=out_ap, in0=ps,
                                            scalar1=bias_ap, scalar2=0.0,
                                            op0=ALU.add, op1=ALU.max)
                evac_ctr[0] += 1

            # ---- conv feature extraction ----
            for i in range(2):
                # layer 1: contraction over the 9 taps (host-built im2col)
                n_t = L_ROWS_CMP[0] // 4
                psums = [psp.tile([128, 4, 126], f32, tag="ps",
                                  name=f"c1p{i}_{rt}") for rt in range(n_t)]
                for rt in range(n_t):
                    nc.tensor.matmul(psums[rt], cw1s,
                                     img9s[:, i, rt * 4:(rt + 1) * 4, 0:126],
                                     start=True, stop=True)
                for rt in range(n_t):
                    evac_relu(psums[rt][0:112],
                              feats[i][0][:, rt * 4:(rt + 1) * 4, 0:126],
                              bias[0:112, 0:1])

                # layers 2..5: 9-tap accumulation, contraction over channels
                for l in range(1, 5):
                    w_out = L_W[l]
                    n_t = L_ROWS_CMP[l] // 4
                    src = feats[i][l - 1]
                    psums = [psp.tile([128, 4, w_out], f32, tag="ps",
                                      name=f"c{l + 1}p{i}_{rt}")
                             for rt in range(n_t)]
                    for t in range(9):
                        dy, dx = divmod(t, 3)
                        lhsT = cws[l - 1][:, t, :]
                        for rt in range(n_t):
                            rhs = src[:, rt * 4 + dy: rt * 4 + 4 + dy,
                                      dx: dx + w_out]
                            nc.tensor.matmul(psums[rt], lhsT, rhs,
                                             start=(t == 0), stop=(t == 8))
                    for rt in range(n_t):
                        if l < 4:
                            dst = feats[i][l][:, rt * 4:(rt + 1) * 4, 0:w_out]
                        else:
                            dst = feats[i][l][:, rt * 4:(rt + 1) * 4, 8:126]
                        evac_relu(psums[rt][0:112], dst, bias[0:112, l:l + 1])

            rf, sf = feats[0][4], feats[1][4]
            ROWB = [(0, 4), (4, 3)]  # (row0, nrows) per matmul block

            # ---- cost-volume MLP, disparities in pairs so consecutive
            # ---- matmuls share the stationary weight tile ----
            D_GROUPS = [[0, 1], [2, 3], [4, 5], [6, 7], [8]]
            for grp in D_GROUPS:
                # blocks: (local d idx, disparity, row0, nrows)
                blks = [(g, dd, r0, nr) for g, dd in enumerate(grp)
                        for (r0, nr) in ROWB]
                gw = 826 * len(grp)

                def off(g, r0):
                    return 826 * g + r0 * 118

                a1 = apool.tile([128, 3, gw], bf, tag="a",
                                name=f"a1_{grp[0]}")
                for m in range(3):
                    ps = [psp.tile([128, nr * 118], f32, tag="ps",
                                   name=f"f1p{grp[0]}_{m}_{b}")
                          for b, (g, dd, r0, nr) in enumerate(blks)]
                    for half in range(2):
                        lhsT = fw1s[:, half, m * 128:(m + 1) * 128]
                        for b, (g, dd, r0, nr) in enumerate(blks):
                            if half == 0:
                                rhs = rf[:, r0:r0 + nr, 8:126]
                            else:
                                sh_ = dd - 8
                                rhs = sf[:, r0:r0 + nr, 8 + sh_:126 + sh_]
                            nc.tensor.matmul(ps[b], lhsT, rhs,
                                             start=(half == 0),
                                             stop=(half == 1))
                    for b, (g, dd, r0, nr) in enumerate(blks):
                        o = off(g, r0)
                        evac_relu(ps[b], a1[:, m, o:o + nr * 118],
                                  bias[:, 5 + m:6 + m])

                a_in = a1
                for li, (fw, bcol) in enumerate(((fw2s, 8), (fw3s, 11))):
                    a_out = apool.tile([128, 3, gw], bf, tag="a",
                                       name=f"a{li + 2}_{grp[0]}")
                    for m in range(3):
                        ps = [psp.tile([128, nr * 118], f32, tag="ps",
                                       name=f"f{li + 2}p{grp[0]}_{m}_{b}")
                              for b, (g, dd, r0, nr) in enumerate(blks)]
                        for c in range(3):
                            lhsT = fw[:, c, m * 128:(m + 1) * 128]
                            for b, (g, dd, r0, nr) in enumerate(blks):
                                o = off(g, r0)
                                nc.tensor.matmul(
                                    ps[b], lhsT, a_in[:, c, o:o + nr * 118],
                                    start=(c == 0), stop=(c == 2))
                        for b, (g, dd, r0, nr) in enumerate(blks):
                            o = off(g, r0)
                            evac_relu(ps[b], a_out[:, m, o:o + nr * 118],
                                      bias[:, bcol + m:bcol + m + 1])
                    a_in = a_out

                # fc4 (384 -> 1) + sigmoid; block-outer so only 1-2 psums live
                stages = {g: stpool.tile([1, 826], f32, tag="stg",
                                         name=f"st{dd_}")
                          for g, dd_ in enumerate(grp)}
                for b, (g, dd, r0, nr) in enumerate(blks):
                    o = off(g, r0)
                    c0 = r0 * 118
                    ps4 = p4p.tile([1, nr * 118], f32, tag="p4",
                                   name=f"f4p{grp[0]}_{b}")
                    for c in range(3):
                        nc.tensor.matmul(ps4, fw4s[:, c:c + 1],
                                         a_in[:, c, o:o + nr * 118],
                                         start=(c == 0), stop=(c == 2))
                    nc.scalar.activation(out=stages[g][:, c0:c0 + nr * 118],
                                         in_=ps4, func=AF.Sigmoid,
                                         bias=fb4s[:, 0:1])
                    if r0 == ROWB[-1][0]:
                        nc.sync.dma_start(out=out_d[dd:dd + 1, :],
                                          in_=stages[g])

    nc.compile()
    return nc


def _get_nc():
    if "nc" not in _prog_cache:
        _prog_cache["nc"] = _build_program()
    return _prog_cache["nc"]


def _bf(x):
    return np.ascontiguousarray(np.asarray(x, np.float32).astype(BF16))


def _pack_shared(conv_ws, conv_bs, fc_ws, fc_bs):
    conv_ws = [np.asarray(w, np.float32) for w in conv_ws]
    conv_bs = [np.asarray(b, np.float32) for b in conv_bs]
    fc_ws = [np.asarray(w, np.float32) for w in fc_ws]
    fc_bs = [np.asarray(b, np.float32) for b in fc_bs]

    sh = {}
    w1 = np.zeros((9, 128), np.float32)
    w1[:, :112] = conv_ws[0].reshape(112, 9).T
    sh["cw1"] = _bf(w1)                                     # [9, 128]
    for l in range(2, 6):
        w = conv_ws[l - 1]                                  # (co, ci, 3, 3)
        wp = np.zeros((112, 9, 128), np.float32)
        wp[:, :, :112] = w.transpose(1, 2, 3, 0).reshape(112, 9, 112)
        sh[f"cw{l}"] = _bf(wp)                              # [ci, tap, co]
    sh["fw1"] = _bf(
        fc_ws[0].reshape(2, 112, 384).transpose(1, 0, 2))   # [k, half, m]
    sh["fw2"] = _bf(
        fc_ws[1].reshape(3, 128, 384).transpose(1, 0, 2))   # [k, chunk, m]
    sh["fw3"] = _bf(
        fc_ws[2].reshape(3, 128, 384).transpose(1, 0, 2))
    sh["fw4"] = _bf(fc_ws[3].reshape(3, 128, 1)[:, :, 0].T)  # [k, chunk]
    bia = np.zeros((128, 14), np.float32)
    for l in range(5):
        bia[:112, l] = conv_bs[l]
    for j, fb in enumerate(fc_bs[:3]):
        bia[:, 5 + 3 * j:8 + 3 * j] = fb.reshape(3, 128).T
    sh["biases"] = bia
    sh["fb4"] = fc_bs[3].reshape(1, 1)
    return sh


def _make_img9(window):
    # window: (17, 128) -> [9, 16, 128] pre-shifted tap copies
    pad = np.zeros((19, 130), np.float32)
    pad[:17, :128] = window
    out = np.empty((9, 16, 128), np.float32)
    for t in range(9):
        dy, dx = divmod(t, 3)
        out[t] = pad[dy:dy + 16, dx:dx + 128]
    return out


_ZEROS = np.zeros((112, 256), BF16)


def _run_cores(ref, sec, sh, trace=False):
    from concourse.bass_utils import run_bass_kernel_spmd
    nc = _get_nc()
    in_maps = []
    for i in range(N_CORES):
        rs = ROW_STARTS[i]
        img9 = np.stack([_make_img9(ref[rs:rs + HALO]),
                         _make_img9(sec[rs:rs + HALO])], axis=1)
        in_maps.append({"img9": _bf(img9), "zeros": _ZEROS, **sh})
    res = run_bass_kernel_spmd(nc, in_maps, list(range(N_CORES)),
                               trace=trace)
    return res


def _gather(results):
    full = np.empty((HF, WF, D), np.float32)
    for i in range(N_CORES):
        o = results[i]["out"].reshape(D, RPC, WF)       # (d, r, x)
        band = -o.transpose(1, 2, 0)                    # (r, x, d)
        rs = ROW_STARTS[i]
        lo = 0 if i == 0 else max(0, RPC * i - rs)
        full[rs + lo: rs + RPC] = band[lo:]
    x = np.arange(WF)[None, :, None]
    ddx = np.arange(D)[None, None, :]
    invalid = (x + (ddx - 8)) < 0
    return np.where(invalid, np.float32(np.nan), full)


def kernel(ref, sec, disp_min, disp_max, conv_ws, conv_bs, fc_ws, fc_bs):
    assert int(disp_min) == -8 and int(disp_max) == 0
    ref = np.asarray(ref, np.float32)
    sec = np.asarray(sec, np.float32)
    sh = _pack_shared(conv_ws, conv_bs, fc_ws, fc_bs)
    res = _run_cores(ref, sec, sh, trace=False)
    return _gather(res.results)


# revision 7
# speedup vs baseline: 1.3072x; 1.3072x over previous
"""MC-CNN accurate-architecture stereo matching cost kernel for Trainium2.

Strategy: data-parallel over image rows across 8 NeuronCores. Each core
computes a 7-row band of the (54, 118, 9) output:
  - conv features for its 17-row input window (7 output rows + 10 halo),
    both images, 5 layers, via tap-accumulated matmuls,
  - the 224->384->384->384->1 MLP for all 9 disparities over its band,
    with disparity shifts realized as SBUF column offsets (no gather),
  - sigmoid on the scalar engine; negation + NaN masking on host.

All matmuls run in bf16 (fp32 PSUM accumulation; ~8e-4 rel error vs the
fp32 reference, simulated host-side). bf16 weights with M=128 enable the
FWL fast weight-load path. Biases are fused into the PSUM-evacuation op
(ACT relu / DVE add+max), alternating engines to balance load.
"""

import ml_dtypes
import numpy as np

BF16 = ml_dtypes.bfloat16

N_CORES = 8
D = 9                 # disparities -8..0
HF, WF = 54, 118
RPC = 7               # output rows per core
HALO = 17             # input rows per core window
ROW_STARTS = [min(RPC * i, HF - RPC) for i in range(N_CORES)]

# conv pipeline geometry (per core): computed rows padded to multiples of 4
L_ROWS_CMP = [16, 16, 12, 12, 8]
L_ROWS_ALL = [18, 16, 14, 12, 8]
L_W = [126, 124, 122, 120, 118]

_prog_cache = {}


def _build_program():
    import concourse.bacc as bacc
    import concourse.mybir as mybir
    import concourse.tile as tile

    f32 = mybir.dt.float32
    bf = mybir.dt.bfloat16
    AF = mybir.ActivationFunctionType
    ALU = mybir.AluOpType

    nc = bacc.Bacc("TRN2", target_bir_lowering=False, debug=False)

    img9_d = nc.declare_dram_parameter("img9", [9, 2, 16, 128], bf, False)
    cw1_d = nc.declare_dram_parameter("cw1", [9, 128], bf, False)
    cw_d = [nc.declare_dram_parameter(f"cw{l}", [112, 9, 128], bf, False)
            for l in (2, 3, 4, 5)]
    fw1_d = nc.declare_dram_parameter("fw1", [112, 2, 384], bf, False)
    fw2_d = nc.declare_dram_parameter("fw2", [128, 3, 384], bf, False)
    fw3_d = nc.declare_dram_parameter("fw3", [128, 3, 384], bf, False)
    fw4_d = nc.declare_dram_parameter("fw4", [128, 3], bf, False)
    bia_d = nc.declare_dram_parameter("biases", [128, 14], f32, False)
    zer_d = nc.declare_dram_parameter("zeros", [112, 256], bf, False)
    fb4_d = nc.declare_dram_parameter("fb4", [1, 1], f32, False)
    out_d = nc.declare_dram_parameter("out", [9, 826], f32, True)

    evac_ctr = [0]

    with tile.TileContext(nc) as tc:
        with tc.tile_pool(name="w", bufs=1) as wp, \
             tc.tile_pool(name="feat", bufs=1) as fp, \
             tc.tile_pool(name="act", bufs=3) as apool, \
             tc.tile_pool(name="stg", bufs=3) as stpool, \
             tc.tile_pool(name="ps", bufs=6, space="PSUM") as psp, \
             tc.tile_pool(name="p4", bufs=2, space="PSUM") as p4p:

            # ---- input / weight DMAs (conv-phase tensors first) ----
            img9s = wp.tile([9, 2, 16, 128], bf)
            for rt in range(4):
                nc.sync.dma_start(out=img9s[:, :, rt * 4:(rt + 1) * 4, :],
                                  in_=img9_d[:, :, rt * 4:(rt + 1) * 4, :])
            cw1s = wp.tile([9, 128], bf)
            nc.sync.dma_start(out=cw1s, in_=cw1_d[:, :])
            cws = []
            for i in range(4):
                t = wp.tile([112, 9, 128], bf, name=f"cw{i + 2}s")
                nc.sync.dma_start(out=t, in_=cw_d[i][:, :, :])
                cws.append(t)
            bias = wp.tile([128, 14], f32)
            nc.sync.dma_start(out=bias, in_=bia_d[:, :])
            fb4s = wp.tile([1, 1], f32)
            nc.sync.dma_start(out=fb4s, in_=fb4_d[:, :])
            fw1s = wp.tile([112, 2, 384], bf)
            nc.sync.dma_start(out=fw1s, in_=fw1_d[:, :, :])
            fw2s = wp.tile([128, 3, 384], bf)
            nc.sync.dma_start(out=fw2s, in_=fw2_d[:, :, :])
            fw3s = wp.tile([128, 3, 384], bf)
            nc.sync.dma_start(out=fw3s, in_=fw3_d[:, :, :])
            fw4s = wp.tile([128, 3], bf)
            nc.sync.dma_start(out=fw4s, in_=fw4_d[:, :])

            # ---- feature tiles ----
            feats = []  # feats[img][layer]
            for i in range(2):
                per = []
                for l in range(5):
                    if l < 4:
                        t = fp.tile([112, L_ROWS_ALL[l], 128], bf,
                                    name=f"f{l + 1}_{i}")
                    else:
                        t = fp.tile([112, 8, 136], bf, name=f"f5_{i}")
                    per.append(t)
                feats.append(per)
                # zero regions that are read before/never being written
                z2 = zer_d[:, :].rearrange("p (a b) -> p a b", a=2)
                z8 = zer_d[:, 0:64].rearrange("p (a b) -> p a b", a=8)
                nc.sync.dma_start(out=per[0][:, 16:18, :], in_=z2)
                nc.sync.dma_start(out=per[2][:, 12:14, :], in_=z2)
                nc.sync.dma_start(out=per[4][:, :, 0:8], in_=z8)

            def evac_relu(ps, out_ap, bias_ap):
                # relu(x + b), PSUM -> SBUF, alternating ACT / DVE
                if evac_ctr[0] % 2 == 0:
                    nc.scalar.activation(out=out_ap, in_=ps, func=AF.Relu,
                                         bias=bias_ap)
                else:
                    nc.vector.tensor_scalar(out=out_ap, in0=ps,
                                            scalar1=bias_ap, scalar2=0.0,
                                            op0=ALU.add, op1=ALU.max)
                evac_ctr[0] += 1

            # ---- conv feature extraction ----
            for i in range(2):
                # layer 1: contraction over the 9 taps (host-built im2col)
                n_t = L_ROWS_CMP[0] // 4
                psums = [psp.tile([128, 4, 126], f32, tag="ps",
                                  name=f"c1p{i}_{rt}") for rt in range(n_t)]
                for rt in range(n_t):
                    nc.tensor.matmul(psums[rt], cw1s,
                                     img9s[:, i, rt * 4:(rt + 1) * 4, 0:126],
                                     start=True, stop=True)
                for rt in range(n_t):
                    evac_relu(psums[rt][0:112],
                              feats[i][0][:, rt * 4:(rt + 1) * 4, 0:126],
                              bias[0:112, 0:1])

                # layers 2..5: 9-tap accumulation, contraction over channels
                for l in range(1, 5):
                    w_out = L_W[l]
                    n_t = L_ROWS_CMP[l] // 4
                    src = feats[i][l - 1]
                    psums = [psp.tile([128, 4, w_out], f32, tag="ps",
                                      name=f"c{l + 1}p{i}_{rt}")
                             for rt in range(n_t)]
                    for t in range(9):
                        dy, dx = divmod(t, 3)
                        lhsT = cws[l - 1][:, t, :]
                        for rt in range(n_t):
                            rhs = src[:, rt * 4 + dy: rt * 4 + 4 + dy,
                                      dx: dx + w_out]
                            nc.tensor.matmul(psums[rt], lhsT, rhs,
                                             start=(t == 0), stop=(t == 8))
                    for rt in range(n_t):
                        if l < 4:
                            dst = feats[i][l][:, rt * 4:(rt + 1) * 4, 0:w_out]
                        else:
                            dst = feats[i][l][:, rt * 4:(rt + 1) * 4, 8:126]
                        evac_relu(psums[rt][0:112], dst, bias[0:112, l:l + 1])

            rf, sf = feats[0][4], feats[1][4]
            ROWB = [(0, 4), (4, 3)]  # (row0, nrows) per matmul block

            # ---- cost-volume MLP, disparities in pairs so consecutive
            # ---- matmuls share the stationary weight tile ----
            D_GROUPS = [[d] for d in range(D)]
            for grp in D_GROUPS:
                # blocks: (local d idx, disparity, row0, nrows)
                blks = [(g, dd, r0, nr) for g, dd in enumerate(grp)
                        for (r0, nr) in ROWB]
                gw = 826 * len(grp)

                def off(g, r0):
                    return 826 * g + r0 * 118

                a1 = apool.tile([128, 3, gw], bf, tag="a",
                                name=f"a1_{grp[0]}")
                for m in range(3):
                    ps = [psp.tile([128, nr * 118], f32, tag="ps",
                                   name=f"f1p{grp[0]}_{m}_{b}")
                          for b, (g, dd, r0, nr) in enumerate(blks)]
                    for half in range(2):
                        lhsT = fw1s[:, half, m * 128:(m + 1) * 128]
                        for b, (g, dd, r0, nr) in enumerate(blks):
                            if half == 0:
                                rhs = rf[:, r0:r0 + nr, 8:126]
                            else:
                                sh_ = dd - 8
                                rhs = sf[:, r0:r0 + nr, 8 + sh_:126 + sh_]
                            nc.tensor.matmul(ps[b], lhsT, rhs,
                                             start=(half == 0),
                                             stop=(half == 1))
                    for b, (g, dd, r0, nr) in enumerate(blks):
                        o = off(g, r0)
                        evac_relu(ps[b], a1[:, m, o:o + nr * 118],
                                  bias[:, 5 + m:6 + m])

                a_in = a1
                for li, (fw, bcol) in enumerate(((fw2s, 8), (fw3s, 11))):
                    a_out = apool.tile([128, 3, gw], bf, tag="a",
                                       name=f"a{li + 2}_{grp[0]}")
                    for m in range(3):
                        ps = [psp.tile([128, nr * 118], f32, tag="ps",
                                       name=f"f{li + 2}p{grp[0]}_{m}_{b}")
                              for b, (g, dd, r0, nr) in enumerate(blks)]
                        for c in range(3):
                            lhsT = fw[:, c, m * 128:(m + 1) * 128]
                            for b, (g, dd, r0, nr) in enumerate(blks):
                                o = off(g, r0)
                                nc.tensor.matmul(
                                    ps[b], lhsT, a_in[:, c, o:o + nr * 118],
                                    start=(c == 0), stop=(c == 2))
                        for b, (g, dd, r0, nr) in enumerate(blks):
                            o = off(g, r0)
                            evac_relu(ps[b], a_out[:, m, o:o + nr * 118],
                                      bias[:, bcol + m:bcol + m + 1])
                    a_in = a_out

                # fc4 (384 -> 1) + sigmoid; block-outer so only 1-2 psums live
                stages = {g: stpool.tile([1, 826], f32, tag="stg",
                                         name=f"st{dd_}")
                          for g, dd_ in enumerate(grp)}
                for b, (g, dd, r0, nr) in enumerate(blks):
                    o = off(g, r0)
                    c0 = r0 * 118
                    ps4 = p4p.tile([1, nr * 118], f32, tag="p4",
                                   name=f"f4p{grp[0]}_{b}")
                    for c in range(3):
                        nc.tensor.matmul(ps4, fw4s[:, c:c + 1],
                                         a_in[:, c, o:o + nr * 118],
                                         start=(c == 0), stop=(c == 2))
                    nc.scalar.activation(out=stages[g][:, c0:c0 + nr * 118],
                                         in_=ps4, func=AF.Sigmoid,
                                         bias=fb4s[:, 0:1])
                    if r0 == ROWB[-1][0]:
                        nc.sync.dma_start(out=out_d[dd:dd + 1, :],
                                          in_=stages[g])

    nc.compile()
    return nc


def _get_nc():
    if "nc" not in _prog_cache:
        _prog_cache["nc"] = _build_program()
    return _prog_cache["nc"]


def _bf(x):
    return np.ascontiguousarray(np.asarray(x, np.float32).astype(BF16))


def _pack_shared(conv_ws, conv_bs, fc_ws, fc_bs):
    conv_ws = [np.asarray(w, np.float32) for w in conv_ws]
    conv_bs = [np.asarray(b, np.float32) for b in conv_bs]
    fc_ws = [np.asarray(w, np.float32) for w in fc_ws]
    fc_bs = [np.asarray(b, np.float32) for b in fc_bs]

    sh = {}
    w1 = np.zeros((9, 128), np.float32)
    w1[:, :112] = conv_ws[0].reshape(112, 9).T
    sh["cw1"] = _bf(w1)                                     # [9, 128]
    for l in range(2, 6):
        w = conv_ws[l - 1]                                  # (co, ci, 3, 3)
        wp = np.zeros((112, 9, 128), np.float32)
        wp[:, :, :112] = w.transpose(1, 2, 3, 0).reshape(112, 9, 112)
        sh[f"cw{l}"] = _bf(wp)                              # [ci, tap, co]
    sh["fw1"] = _bf(
        fc_ws[0].reshape(2, 112, 384).transpose(1, 0, 2))   # [k, half, m]
    sh["fw2"] = _bf(
        fc_ws[1].reshape(3, 128, 384).transpose(1, 0, 2))   # [k, chunk, m]
    sh["fw3"] = _bf(
        fc_ws[2].reshape(3, 128, 384).transpose(1, 0, 2))
    sh["fw4"] = _bf(fc_ws[3].reshape(3, 128, 1)[:, :, 0].T)  # [k, chunk]
    bia = np.zeros((128, 14), np.float32)
    for l in range(5):
        bia[:112, l] = conv_bs[l]
    for j, fb in enumerate(fc_bs[:3]):
        bia[:, 5 + 3 * j:8 + 3 * j] = fb.reshape(3, 128).T
    sh["biases"] = bia
    sh["fb4"] = fc_bs[3].reshape(1, 1)
    return sh


def _make_img9(window):
    # window: (17, 128) -> [9, 16, 128] pre-shifted tap copies
    pad = np.zeros((19, 130), np.float32)
    pad[:17, :128] = window
    out = np.empty((9, 16, 128), np.float32)
    for t in range(9):
        dy, dx = divmod(t, 3)
        out[t] = pad[dy:dy + 16, dx:dx + 128]
    return out


_ZEROS = np.zeros((112, 256), BF16)


def _run_cores(ref, sec, sh, trace=False):
    from concourse.bass_utils import run_bass_kernel_spmd
    nc = _get_nc()
    in_maps = []
    for i in range(N_CORES):
        rs = ROW_STARTS[i]
        img9 = np.stack([_make_img9(ref[rs:rs + HALO]),
                         _make_img9(sec[rs:rs + HALO])], axis=1)
        in_maps.append({"img9": _bf(img9), "zeros": _ZEROS, **sh})
    res = run_bass_kernel_spmd(nc, in_maps, list(range(N_CORES)),
                               trace=trace)
    return res


def _gather(results):
    full = np.empty((HF, WF, D), np.float32)
    for i in range(N_CORES):
        o = results[i]["out"].reshape(D, RPC, WF)       # (d, r, x)
        band = -o.transpose(1, 2, 0)                    # (r, x, d)
        rs = ROW_STARTS[i]
        lo = 0 if i == 0 else max(0, RPC * i - rs)
        full[rs + lo: rs + RPC] = band[lo:]
    x = np.arange(WF)[None, :, None]
    ddx = np.arange(D)[None, None, :]
    invalid = (x + (ddx - 8)) < 0
    return np.where(invalid, np.float32(np.nan), full)


def kernel(ref, sec, disp_min, disp_max, conv_ws, conv_bs, fc_ws, fc_bs):
    assert int(disp_min) == -8 and int(disp_max) == 0
    ref = np.asarray(ref, np.float32)
    sec = np.asarray(sec, np.float32)
    sh = _pack_shared(conv_ws, conv_bs, fc_ws, fc_bs)
    res = _run_cores(ref, sec, sh, trace=False)
    return _gather(res.results)


# revision 8
# speedup vs baseline: 1.3685x; 1.0469x over previous
"""MC-CNN accurate-architecture stereo matching cost kernel for Trainium2.

Strategy: data-parallel over image rows across 8 NeuronCores. Each core
computes a 7-row band of the (54, 118, 9) output:
  - conv features for its 17-row input window (7 output rows + 10 halo),
    both images, 5 layers, via tap-accumulated matmuls,
  - the 224->384->384->384->1 MLP for all 9 disparities over its band,
    with disparity shifts realized as SBUF column offsets (no gather),
  - sigmoid on the scalar engine; negation + NaN masking on host.

All matmuls run in bf16 (fp32 PSUM accumulation; ~8e-4 rel error vs the
fp32 reference, simulated host-side). bf16 weights with M=128 enable the
FWL fast weight-load path. Biases are fused into the PSUM-evacuation op
(ACT relu / DVE add+max), alternating engines to balance load.
"""

import ml_dtypes
import numpy as np

BF16 = ml_dtypes.bfloat16

N_CORES = 8
D = 9                 # disparities -8..0
HF, WF = 54, 118
RPC = 7               # output rows per core
HALO = 17             # input rows per core window
ROW_STARTS = [min(RPC * i, HF - RPC) for i in range(N_CORES)]

# conv pipeline geometry (per core): computed rows padded to multiples of 4
L_ROWS_CMP = [16, 16, 12, 12, 8]
L_ROWS_ALL = [18, 16, 14, 12, 8]
L_W = [126, 124, 122, 120, 118]

_prog_cache = {}


def _build_program():
    import concourse.bacc as bacc
    import concourse.mybir as mybir
    import concourse.tile as tile

    f32 = mybir.dt.float32
    bf = mybir.dt.bfloat16
    AF = mybir.ActivationFunctionType
    ALU = mybir.AluOpType

    nc = bacc.Bacc("TRN2", target_bir_lowering=False, debug=False)

    img9_d = nc.declare_dram_parameter("img9", [9, 2, 16, 128], bf, False)
    cw1_d = nc.declare_dram_parameter("cw1", [9, 128], bf, False)
    cw_d = [nc.declare_dram_parameter(f"cw{l}", [112, 9, 128], bf, False)
            for l in (2, 3, 4, 5)]
    fw1_d = nc.declare_dram_parameter("fw1", [112, 2, 384], bf, False)
    fw2_d = nc.declare_dram_parameter("fw2", [128, 3, 384], bf, False)
    fw3_d = nc.declare_dram_parameter("fw3", [128, 3, 384], bf, False)
    fw4_d = nc.declare_dram_parameter("fw4", [128, 3], bf, False)
    bia_d = nc.declare_dram_parameter("biases", [128, 14], f32, False)
    zer_d = nc.declare_dram_parameter("zeros", [112, 256], bf, False)
    fb4_d = nc.declare_dram_parameter("fb4", [1, 1], f32, False)
    out_d = nc.declare_dram_parameter("out", [9, 826], f32, True)

    evac_ctr = [0]

    with tile.TileContext(nc) as tc:
        with tc.tile_pool(name="w", bufs=1) as wp, \
             tc.tile_pool(name="feat", bufs=1) as fp, \
             tc.tile_pool(name="act", bufs=3) as apool, \
             tc.tile_pool(name="stg", bufs=3) as stpool, \
             tc.tile_pool(name="ps", bufs=6, space="PSUM") as psp, \
             tc.tile_pool(name="p4", bufs=2, space="PSUM") as p4p:

            # ---- input / weight DMAs (conv-phase tensors first) ----
            img9s = wp.tile([9, 2, 16, 128], bf)
            for rt in range(4):
                nc.sync.dma_start(out=img9s[:, :, rt * 4:(rt + 1) * 4, :],
                                  in_=img9_d[:, :, rt * 4:(rt + 1) * 4, :])
            cw1s = wp.tile([9, 128], bf)
            nc.sync.dma_start(out=cw1s, in_=cw1_d[:, :])
            cws = []
            for i in range(4):
                t = wp.tile([112, 9, 128], bf, name=f"cw{i + 2}s")
                nc.sync.dma_start(out=t, in_=cw_d[i][:, :, :])
                cws.append(t)
            bias = wp.tile([128, 14], f32)
            nc.sync.dma_start(out=bias, in_=bia_d[:, :])
            fb4s = wp.tile([1, 1], f32)
            nc.sync.dma_start(out=fb4s, in_=fb4_d[:, :])
            fw1s = wp.tile([112, 2, 384], bf)
            nc.sync.dma_start(out=fw1s, in_=fw1_d[:, :, :])
            fw2s = wp.tile([128, 3, 384], bf)
            nc.sync.dma_start(out=fw2s, in_=fw2_d[:, :, :])
            fw3s = wp.tile([128, 3, 384], bf)
            nc.sync.dma_start(out=fw3s, in_=fw3_d[:, :, :])
            fw4s = wp.tile([128, 3], bf)
            nc.sync.dma_start(out=fw4s, in_=fw4_d[:, :])

            # ---- feature tiles ----
            feats = []  # feats[img][layer]
            for i in range(2):
                per = []
                for l in range(5):
                    if l < 4:
                        t = fp.tile([112, L_ROWS_ALL[l], 128], bf,
                                    name=f"f{l + 1}_{i}")
                    else:
                        t = fp.tile([112, 8, 136], bf, name=f"f5_{i}")
                    per.append(t)
                feats.append(per)
                # zero regions that are read before/never being written
                z2 = zer_d[:, :].rearrange("p (a b) -> p a b", a=2)
                z8 = zer_d[:, 0:64].rearrange("p (a b) -> p a b", a=8)
                nc.sync.dma_start(out=per[0][:, 16:18, :], in_=z2)
                nc.sync.dma_start(out=per[2][:, 12:14, :], in_=z2)
                nc.sync.dma_start(out=per[4][:, :, 0:8], in_=z8)

            def evac_relu(ps, out_ap, bias_ap):
                # relu(x + b), PSUM -> SBUF, alternating ACT / DVE
                if evac_ctr[0] % 2 == 0:
                    nc.scalar.activation(out=out_ap, in_=ps, func=AF.Relu,
                                         bias=bias_ap)
                else:
                    nc.vector.tensor_scalar(out=out_ap, in0=ps,
                                            scalar1=bias_ap, scalar2=0.0,
                                            op0=ALU.add, op1=ALU.max)
                evac_ctr[0] += 1

            # ---- conv feature extraction ----
            for i in range(2):
                # layer 1: contraction over the 9 taps (host-built im2col)
                n_t = L_ROWS_CMP[0] // 4
                psums = [psp.tile([128, 4, 126], f32, tag="ps",
                                  name=f"c1p{i}_{rt}") for rt in range(n_t)]
                for rt in range(n_t):
                    nc.tensor.matmul(psums[rt], cw1s,
                                     img9s[:, i, rt * 4:(rt + 1) * 4, 0:126],
                                     start=True, stop=True)
                for rt in range(n_t):
                    evac_relu(psums[rt][0:112],
                              feats[i][0][:, rt * 4:(rt + 1) * 4, 0:126],
                              bias[0:112, 0:1])

                # layers 2..5: 9-tap accumulation, contraction over channels
                for l in range(1, 5):
                    w_out = L_W[l]
                    n_t = L_ROWS_CMP[l] // 4
                    src = feats[i][l - 1]
                    psums = [psp.tile([128, 4, w_out], f32, tag="ps",
                                      name=f"c{l + 1}p{i}_{rt}")
                             for rt in range(n_t)]
                    for t in range(9):
                        dy, dx = divmod(t, 3)
                        lhsT = cws[l - 1][:, t, :]
                        for rt in range(n_t):
                            rhs = src[:, rt * 4 + dy: rt * 4 + 4 + dy,
                                      dx: dx + w_out]
                            nc.tensor.matmul(psums[rt], lhsT, rhs,
                                             start=(t == 0), stop=(t == 8))
                    for rt in range(n_t):
                        if l < 4:
                            dst = feats[i][l][:, rt * 4:(rt + 1) * 4, 0:w_out]
                        else:
                            dst = feats[i][l][:, rt * 4:(rt + 1) * 4, 8:126]
                        evac_relu(psums[rt][0:112], dst, bias[0:112, l:l + 1])

            rf, sf = feats[0][4], feats[1][4]
            ROWB = [(0, 4), (4, 3)]  # (row0, nrows) per matmul block

            # ---- fc1, disparity-invariant halves ----
            # fc1(d) = relu(W1a@rf[x] + W1b@sf[x+d] + b1): compute
            # yref = W1a@rf and ysec = W1b@sf + b1 once; each disparity
            # is then a shifted elementwise add + relu.
            yref = fp.tile([128, 3, 7, 118], f32)
            ysec = fp.tile([128, 3, 7, 126], f32)
            for m in range(3):
                for half, (dst, lo, hi) in enumerate(
                        ((yref, 8, 126), (ysec, 0, 126))):
                    w_ = fw1s[:, half, m * 128:(m + 1) * 128]
                    for r0, nr in ROWB:
                        wd = hi - lo
                        ps = psp.tile([128, nr * wd], f32, tag="ps",
                                      name=f"y{half}_{m}_{r0}")
                        src_ = rf if half == 0 else sf
                        nc.tensor.matmul(ps, w_, src_[:, r0:r0 + nr, lo:hi],
                                         start=True, stop=True)
                        out_ap = dst[:, m, r0:r0 + nr, :]
                        if half == 0:
                            nc.scalar.copy(out=out_ap, in_=ps)
                        else:
                            nc.vector.tensor_scalar(
                                out=out_ap, in0=ps,
                                scalar1=bias[:, 5 + m:6 + m], scalar2=None,
                                op0=ALU.add)

            # ---- cost-volume MLP over 3-disparity groups ----
            D_GROUPS = [[0, 1, 2], [3, 4, 5], [6, 7, 8]]
            for grp in D_GROUPS:
                gw = 826 * len(grp)
                # a1 = relu(yref + shift_d(ysec)), one add+relu per (d, m)
                a1 = apool.tile([128, 3, gw], bf, tag="a",
                                name=f"a1_{grp[0]}")
                for g, dd in enumerate(grp):
                    sh_ = dd - 8
                    for m in range(3):
                        o = 826 * g
                        dst = a1[:, m, o:o + 826].rearrange(
                            "p (r x) -> p r x", r=7)
                        tmp = apool.tile([128, 7, 118], f32, tag="tmp",
                                         name=f"t{dd}_{m}")
                        nc.vector.tensor_tensor(
                            out=tmp, in0=yref[:, m],
                            in1=ysec[:, m, :, 8 + sh_:126 + sh_],
                            op=ALU.add)
                        if (dd + m) % 2 == 0:
                            nc.scalar.activation(out=dst, in_=tmp,
                                                 func=AF.Relu)
                        else:
                            nc.vector.tensor_scalar(out=dst, in0=tmp,
                                                    scalar1=0.0, scalar2=None,
                                                    op0=ALU.max)

                # fc2, fc3 over 512-wide chunks of the whole group
                CHUNKS = []
                x0 = 0
                while x0 < gw:
                    CHUNKS.append((x0, min(512, gw - x0)))
                    x0 += 512
                a_in = a1
                for li, (fw, bcol) in enumerate(((fw2s, 8), (fw3s, 11))):
                    a_out = apool.tile([128, 3, gw], bf, tag="a",
                                       name=f"a{li + 2}_{grp[0]}")
                    for m in range(3):
                        for x0, nx in CHUNKS:
                            ps = psp.tile([128, nx], f32, tag="ps",
                                          name=f"f{li}p{grp[0]}_{m}_{x0}")
                            for c in range(3):
                                nc.tensor.matmul(
                                    ps, fw[:, c, m * 128:(m + 1) * 128],
                                    a_in[:, c, x0:x0 + nx],
                                    start=(c == 0), stop=(c == 2))
                            evac_relu(ps, a_out[:, m, x0:x0 + nx],
                                      bias[:, bcol + m:bcol + m + 1])
                    a_in = a_out

                # fc4 (384 -> 1) + sigmoid
                stage = stpool.tile([1, gw], f32, tag="stg",
                                    name=f"st{grp[0]}")
                for x0, nx in CHUNKS:
                    ps4 = p4p.tile([1, nx], f32, tag="p4",
                                   name=f"f4p{grp[0]}_{x0}")
                    for c in range(3):
                        nc.tensor.matmul(ps4, fw4s[:, c:c + 1],
                                         a_in[:, c, x0:x0 + nx],
                                         start=(c == 0), stop=(c == 2))
                    nc.scalar.activation(out=stage[:, x0:x0 + nx],
                                         in_=ps4, func=AF.Sigmoid,
                                         bias=fb4s[:, 0:1])
                for g, dd in enumerate(grp):
                    nc.sync.dma_start(out=out_d[dd:dd + 1, :],
                                      in_=stage[:, 826 * g:826 * (g + 1)])

    nc.compile()
    return nc


def _get_nc():
    if "nc" not in _prog_cache:
        _prog_cache["nc"] = _build_program()
    return _prog_cache["nc"]


def _bf(x):
    return np.ascontiguousarray(np.asarray(x, np.float32).astype(BF16))


def _pack_shared(conv_ws, conv_bs, fc_ws, fc_bs):
    conv_ws = [np.asarray(w, np.float32) for w in conv_ws]
    conv_bs = [np.asarray(b, np.float32) for b in conv_bs]
    fc_ws = [np.asarray(w, np.float32) for w in fc_ws]
    fc_bs = [np.asarray(b, np.float32) for b in fc_bs]

    sh = {}
    w1 = np.zeros((9, 128), np.float32)
    w1[:, :112] = conv_ws[0].reshape(112, 9).T
    sh["cw1"] = _bf(w1)                                     # [9, 128]
    for l in range(2, 6):
        w = conv_ws[l - 1]                                  # (co, ci, 3, 3)
        wp = np.zeros((112, 9, 128), np.float32)
        wp[:, :, :112] = w.transpose(1, 2, 3, 0).reshape(112, 9, 112)
        sh[f"cw{l}"] = _bf(wp)                              # [ci, tap, co]
    sh["fw1"] = _bf(
        fc_ws[0].reshape(2, 112, 384).transpose(1, 0, 2))   # [k, half, m]
    sh["fw2"] = _bf(
        fc_ws[1].reshape(3, 128, 384).transpose(1, 0, 2))   # [k, chunk, m]
    sh["fw3"] = _bf(
        fc_ws[2].reshape(3, 128, 384).transpose(1, 0, 2))
    sh["fw4"] = _bf(fc_ws[3].reshape(3, 128, 1)[:, :, 0].T)  # [k, chunk]
    bia = np.zeros((128, 14), np.float32)
    for l in range(5):
        bia[:112, l] = conv_bs[l]
    for j, fb in enumerate(fc_bs[:3]):
        bia[:, 5 + 3 * j:8 + 3 * j] = fb.reshape(3, 128).T
    sh["biases"] = bia
    sh["fb4"] = fc_bs[3].reshape(1, 1)
    return sh


def _make_img9(window):
    # window: (17, 128) -> [9, 16, 128] pre-shifted tap copies
    pad = np.zeros((19, 130), np.float32)
    pad[:17, :128] = window
    out = np.empty((9, 16, 128), np.float32)
    for t in range(9):
        dy, dx = divmod(t, 3)
        out[t] = pad[dy:dy + 16, dx:dx + 128]
    return out


_ZEROS = np.zeros((112, 256), BF16)


def _run_cores(ref, sec, sh, trace=False):
    from concourse.bass_utils import run_bass_kernel_spmd
    nc = _get_nc()
    in_maps = []
    for i in range(N_CORES):
        rs = ROW_STARTS[i]
        img9 = np.stack([_make_img9(ref[rs:rs + HALO]),
                         _make_img9(sec[rs:rs + HALO])], axis=1)
        in_maps.append({"img9": _bf(img9), "zeros": _ZEROS, **sh})
    res = run_bass_kernel_spmd(nc, in_maps, list(range(N_CORES)),
                               trace=trace)
    return res


def _gather(results):
    full = np.empty((HF, WF, D), np.float32)
    for i in range(N_CORES):
        o = results[i]["out"].reshape(D, RPC, WF)       # (d, r, x)
        band = -o.transpose(1, 2, 0)                    # (r, x, d)
        rs = ROW_STARTS[i]
        lo = 0 if i == 0 else max(0, RPC * i - rs)
        full[rs + lo: rs + RPC] = band[lo:]
    x = np.arange(WF)[None, :, None]
    ddx = np.arange(D)[None, None, :]
    invalid = (x + (ddx - 8)) < 0
    return np.where(invalid, np.float32(np.nan), full)


def kernel(ref, sec, disp_min, disp_max, conv_ws, conv_bs, fc_ws, fc_bs):
    assert int(disp_min) == -8 and int(disp_max) == 0
    ref = np.asarray(ref, np.float32)
    sec = np.asarray(sec, np.float32)
    sh = _pack_shared(conv_ws, conv_bs, fc_ws, fc_bs)
    res = _run_cores(ref, sec, sh, trace=False)
    return _gather(res.results)
